# revision 17
# baseline (speedup 1.0000x reference)
"""Trainium2 Bass kernel for CausalGNNLayer (per-node-type Linear, MoE-style routing).

Semantics (matching the reference):
    out[n, :] = x[n, :] @ W[node_types[n]].T + b[node_types[n]]
edge_index is unused by the op.

Strategy (v2 — fp8e4 DoubleRow, P0-downclock-aware):
- Host-side routing-aware sharding: stable-sort nodes by type, split each
  type's node list into two halves -> 8 groups (4 types x 2 cores).
- Measured on this part: a sustained 8-core bf16-PE stream trips the chip's
  P0 power state and pins the PE at 2.0 GHz (259 ns / 512-wide matmul);
  an fp8e4 DoubleRow stream (2 MACs/PE/cycle, K=256 per instruction) stays
  at 2.4 GHz (216 ns).  DoubleRow needs both operands in fp8-e4m3, whose
  3-bit mantissa alone would blow the 2e-2 gate (measured 3.8e-2), so we
  run TWO DoubleRow streams accumulated in one PSUM group:
      y*2^17 = m1 @ Wq + u @ V,  where
      m1 = Q((1-a)*x*32),  xhat = m1/((1-a)*32),  xlo = x - xhat
      u  = Q((xlo + a*xhat)*32)
      Wq = Q(W*4096),      V = Q((What + (W-What)/a)*4096),  a = 1/8
  The correction stream cancels both operands' quantization error to first
  order: measured end-to-end rel err 7.2e-3 (vs 1.18e-2 for the old
  bf16xfp8e3 kernel).  Same instruction count as the bf16 schedule, but at
  216 ns/instr instead of 259: steady-state ~85 us vs ~102 us.
  Scales are powers of two (32*4096 = 2^17); bias is pre-scaled by 2^17 on
  host and the output divided by 2^17 after the run, so the device drain is
  still a plain add+downcast.
- Group-major tiles: 1024 nodes per x tile / psum burst / output tile, ONE
  dma_start in and ONE out per group (each dma_start costs ~700 ns of sync-
  queue DIRECT2D issue; the old 4-outs-per-group schedule kept the sync
  queue 94% busy and serialized the tail).
- out2 dram layout [128, 4, P] (partition-major) so the 4 psum drains of a
  group land in one SBUF tile and leave as one strided DMA.
- Warmup: dummy DoubleRow matmuls on a zeroed tile ramp the HAM clock gate
  (cold 1.2 GHz for ~3.4 us) while the first data DMAs land; mini 128-node
  first chunk starts real work on ~72 KB of data.
- Drain (bias add + fp32->bf16 downcast) alternates Vector/Scalar engines.
- Host scatters the 8 output shards back into the full [N, 512] fp32 output.
"""

import numpy as np
import ml_dtypes
from contextlib import ExitStack

import concourse.bass as bass
import concourse.mybir as mybir
import concourse.tile as tile
from concourse.bass_utils import run_bass_kernel_spmd

N_CORES = 8
IN_CH = 512
OUT_CH = 512
NUM_TYPES = 4
P_BLK = 128
OBLKS = OUT_CH // P_BLK   # 4
MINI_N = 128              # first chunk width (early compute start)
MID_N = 512               # second chunk width (bridges to the 1MB groups)
GRP_N = 1024              # steady group width (one x tile / one out tile)
XBUFS = 4                 # x group-tile prefetch depth (8KB/partition each)
PSBUFS = 4                # psum ring: 4 tiles x 2 banks = all 8 banks
OBUFS = 2                 # output staging depth (8KB/partition each)
TAIL_N = 256              # last chunk width (bounds the exposed final DMA)
WARMUP_MM = 12            # dummy DoubleRow matmuls to ramp the clock gate

ALPHA = 0.125
SM = 32.0                 # moving-operand scale
SW = 4096.0               # stationary-operand scale
SCALE = SM * SW           # 2^17

TRACE = False
LAST_RESULTS = None

_compile_cache: dict = {}

_legal_nop_counter = [0]


def _legalize_waits(nc: bass.Bass) -> None:
    """This walrus codegen only encodes ONE sync wait per engine instruction.
    Tile's scheduler attaches several.  Split: hoist all-but-one wait of any
    multi-wait instruction into preceding same-engine NoOps (one wait each) —
    semantically identical (the engine stalls on each wait in program order)."""
    for fn in nc.m.functions:
        for blk in fn.blocks:
            insts = blk.instructions
            out = []
            changed = False
            for inst in insts:
                si = inst.sync_info
                waits = list(si.on_wait) if si is not None and si.on_wait else []
                if len(waits) > 1:
                    changed = True
                    for w in waits[:-1]:
                        _legal_nop_counter[0] += 1
                        nop = mybir.InstNoOp(
                            name=f"waitsplit-{_legal_nop_counter[0]}",
                            ins=[],
                            outs=[],
                            engine=inst.engine,
                        )
                        nop.sync_info = mybir.SyncInfo(on_wait=[w], on_update=[])
                        out.append(nop)
                    inst.sync_info = mybir.SyncInfo(
                        on_wait=[waits[-1]], on_update=list(si.on_update or [])
                    )
                out.append(inst)
            if changed:
                blk.instructions = out


def _plan(P_needed: int):
    """Chunk widths [128, 512, 1024, ..., 1024, final] covering >= P_needed.
    Graded early chunks: the 128-node mini needs only 128KB of DMA (first
    data to land, ~13us after kernel start under 8-core HBM contention),
    the 512 bridges while the 1MB group tiles stream in."""
    rem = P_needed - MINI_N - MID_N - TAIL_N
    nfull = max(0, (rem - 1) // GRP_N)
    mid2 = rem - nfull * GRP_N
    mid2 = ((mid2 + 63) // 64) * 64
    # remainder group sits second-to-last; the fixed small TAIL_N group ends
    # the kernel so the last exposed out-DMA is only ~256KB.
    widths = (
        [MINI_N, MID_N] + [GRP_N] * nfull + ([mid2] if mid2 else []) + [TAIL_N]
    )
    offs = np.concatenate([[0], np.cumsum(widths)]).astype(int)
    return widths, list(offs[:-1]), int(offs[-1])


def _build_bass(plan_key) -> bass.Bass:
    widths, offs, P = plan_key
    nc = bass.Bass("TRN2")
    f32 = mybir.dt.float32
    bf16 = mybir.dt.bfloat16
    f8e4 = mybir.dt.float8e4
    DR = mybir.MatmulPerfMode.DoubleRow

    nchunks = len(widths)

    # xT[c, p, s, j, i, n]: stream s, kk-pair j, plane i, node n of chunk c;
    # contraction index kappa = j*256 + i*128 + p.
    xT = nc.dram_tensor(
        "xT", [nchunks, P_BLK, 2, 2, 2, GRP_N], f8e4, kind="ExternalInput"
    )
    # w8[p, s, j, i, o*128+m]
    w8 = nc.dram_tensor("w8", [P_BLK, 2, 2, 2, OUT_CH], f8e4, kind="ExternalInput")
    # bias2[p, oblk] = b[oblk*128 + p] * 2^17
    bias2 = nc.dram_tensor("bias2", [P_BLK, OBLKS], f32, kind="ExternalInput")
    # out2[p, oblk, n] = (y[n, oblk*128+p] * 2^17) as bf16
    out2 = nc.dram_tensor("out2", [P_BLK, OBLKS, P], bf16, kind="ExternalOutput")

    with ExitStack() as ctx:
        tc = ctx.enter_context(tile.TileContext(nc))
        sp = ctx.enter_context(tc.tile_pool(name="st", bufs=3))
        xp = ctx.enter_context(tc.tile_pool(name="x", bufs=XBUFS))
        pp = ctx.enter_context(tc.tile_pool(name="ps", bufs=PSBUFS, space="PSUM"))
        op = ctx.enter_context(tc.tile_pool(name="o", bufs=OBUFS))

        # Clock-gate warmup: dummy DoubleRow matmuls on zeros, no DMA deps.
        warm_sb = sp.tile([P_BLK, 2, 512], f8e4)
        nc.gpsimd.memset(warm_sb[:], 0)
        ps_warm = pp.tile([P_BLK, 512], f32, tag="ps")
        for _ in range(WARMUP_MM):
            nc.tensor.matmul(
                ps_warm[:],
                lhsT=warm_sb[:, :, 0:P_BLK],
                rhs=warm_sb[:],
                start=True,
                stop=True,
                perf_mode=DR,
            )

        x_tiles: dict[int, object] = {}

        def fetch_chunk(c: int):
            if c not in x_tiles:
                wd = widths[c]
                t = xp.tile([P_BLK, 2, 2, 2, wd], f8e4, tag="x")
                nc.sync.dma_start(t[:], xT.ap()[c][:, :, :, :, 0:wd])
                x_tiles[c] = t

        w_sb = sp.tile([P_BLK, 2, 2, 2, OUT_CH], f8e4)
        # x-in DMAs issue on the sync queue; w/bias/out on the scalar queue so
        # the two DIRECT2D streams (~0.7us each to issue) run in parallel and
        # the tail's final out-DMA doesn't queue behind x prefetches.
        nc.sync.dma_start(w_sb[:, 0], w8.ap()[:, 0])
        fetch_chunk(0)
        if len(widths) > 1:
            fetch_chunk(1)
        nc.sync.dma_start(w_sb[:, 1], w8.ap()[:, 1])
        b_sb = sp.tile([P_BLK, OBLKS], f32)
        nc.sync.dma_start(b_sb[:], bias2.ap())

        drain_flip = [0]

        def drain(o_sb, oblk, ps_ap, force_vector=False):
            bias_ap = b_sb[:, oblk: oblk + 1]
            if force_vector or drain_flip[0] % 2 == 0:
                nc.vector.tensor_scalar_add(o_sb[:, oblk, :], ps_ap, bias_ap)
            else:
                nc.scalar.add(o_sb[:, oblk, :], ps_ap, bias_ap)
            drain_flip[0] += 1

        for c in range(nchunks):
            fetch_chunk(c)
            for cn in range(c + 1, min(c + 3, nchunks)):
                fetch_chunk(cn)
            wd = widths[c]
            goff = offs[c]
            xt = x_tiles[c]
            o_sb = op.tile([P_BLK, OBLKS, wd], bf16, tag="o")
            if c == 0:
                # Mini chunk: (s,j)-outer over all 4 oblk psum slices so the
                # second w half gets extra time to land.  Slices sit at
                # 512-element offsets (psum-bank aligned).
                psA = pp.tile([P_BLK, 2 * 512], f32, tag="ps")
                psB = pp.tile([P_BLK, 2 * 512], f32, tag="ps")
                mslice = lambda oblk: (psA if oblk < 2 else psB)[
                    :, (oblk % 2) * 512: (oblk % 2) * 512 + wd
                ]
                for s in range(2):
                    for j in range(2):
                        for oblk in range(OBLKS):
                            nc.tensor.matmul(
                                mslice(oblk),
                                lhsT=w_sb[:, s, j, :, oblk * P_BLK:(oblk + 1) * P_BLK],
                                rhs=xt[:, s, j, :, 0:wd],
                                start=(s == 0 and j == 0),
                                stop=(s == 1 and j == 1),
                                perf_mode=DR,
                            )
                for oblk in range(OBLKS):
                    drain(o_sb, oblk, mslice(oblk))
            else:
                for oblk in range(OBLKS):
                    ps = pp.tile([P_BLK, wd], f32, tag="ps")
                    for s in range(2):
                        for j in range(2):
                            lhsT = w_sb[:, s, j, :, oblk * P_BLK:(oblk + 1) * P_BLK]
                            for h in range(0, wd, 512):
                                he = min(h + 512, wd)
                                nc.tensor.matmul(
                                    ps[:, h:he],
                                    lhsT=lhsT,
                                    rhs=xt[:, s, j, :, h:he],
                                    start=(s == 0 and j == 0),
                                    stop=(s == 1 and j == 1),
                                    perf_mode=DR,
                                )
                    drain(o_sb, oblk, ps[:], force_vector=(c == nchunks - 1))
            nc.sync.dma_start(out2.ap()[:, :, goff:goff + wd], o_sb[:])
    _legalize_waits(nc)
    return nc


def _get_compiled(plan_key) -> bass.Bass:
    key = (tuple(plan_key[0]), plan_key[2])
    if key not in _compile_cache:
        _compile_cache[key] = _build_bass(plan_key)
    return _compile_cache[key]


def _qe4(a):
    return np.clip(a, -224.0, 224.0).astype(ml_dtypes.float8_e4m3)


def kernel(x, edge_index, node_types, W, b):
    global LAST_RESULTS
    x = np.asarray(x, dtype=np.float32)
    nt = np.asarray(node_types).astype(np.int64)
    W = np.asarray(W, dtype=np.float32)
    b = np.asarray(b, dtype=np.float32)
    N = x.shape[0]

    # Route nodes: stable sort by type, split each type across 2 cores.
    order = np.argsort(nt, kind="stable")
    counts = np.bincount(nt, minlength=NUM_TYPES)
    shards = []
    start = 0
    for t in range(NUM_TYPES):
        c = int(counts[t])
        idx = order[start: start + c]
        start += c
        h = (c + 1) // 2
        shards.append(idx[:h])
        shards.append(idx[h:])

    P_needed = max(1, max(len(g) for g in shards))
    plan = _plan(P_needed)
    widths, offs, P = plan
    nchunks = len(widths)

    nc = _get_compiled(plan)

    # Per-type quantized weights (shared by the 2 cores of each type).
    w_packed = []
    b_packed = []
    for t in range(NUM_TYPES):
        Wq = _qe4(W[t] * SW)                      # [O, K] e4m3
        What = Wq.astype(np.float32) / SW
        V = _qe4((What + (W[t] - What) / ALPHA) * SW)
        # [p, s, j, i, o]: Wq/V [o, kappa] -> kappa = j*256 + i*128 + p
        wp = np.empty((P_BLK, 2, 2, 2, OUT_CH), ml_dtypes.float8_e4m3)
        for s, Ws in enumerate((Wq, V)):
            # Ws.T [K, O] -> [j, i, p, O]
            wt = Ws.T.reshape(2, 2, P_BLK, OUT_CH)
            wp[:, s] = wt.transpose(2, 0, 1, 3)
        w_packed.append(wp)
        b_packed.append(
            np.ascontiguousarray((b[t] * SCALE).reshape(OBLKS, P_BLK).T.astype(np.float32))
        )

    in_maps = []
    for gi, g in enumerate(shards):
        t = gi // 2
        xs = np.zeros((P, IN_CH), np.float32)
        if len(g):
            xs[: len(g)] = x[g]
        m1 = _qe4((1 - ALPHA) * xs * SM)
        xhat = m1.astype(np.float32) / ((1 - ALPHA) * SM)
        u = _qe4((xs - xhat + ALPHA * xhat) * SM)
        xbuf = np.zeros((nchunks, P_BLK, 2, 2, 2, GRP_N), ml_dtypes.float8_e4m3)
        for s, Xs in enumerate((m1, u)):
            XsT = Xs.T.reshape(2, 2, P_BLK, P)  # [j, i, p, node]
            for c in range(nchunks):
                wd = widths[c]
                xbuf[c, :, s, :, :, :wd] = XsT[
                    :, :, :, offs[c]: offs[c] + wd
                ].transpose(2, 0, 1, 3)
        in_maps.append({"xT": xbuf, "w8": w_packed[t], "bias2": b_packed[t]})

    res = run_bass_kernel_spmd(nc, in_maps, list(range(N_CORES)), trace=TRACE)
    LAST_RESULTS = res

    inv = np.float32(1.0 / SCALE)
    out = np.empty((N, OUT_CH), np.float32)
    for gi, g in enumerate(shards):
        if len(g):
            # out2 [128, 4, P] -> [P, 512] node-major (channel = oblk*128+p)
            o = res.results[gi]["out2"].astype(np.float32).transpose(2, 1, 0)
            out[g] = o.reshape(P, OUT_CH)[: len(g)] * inv
    return out


# revision 19
# speedup vs baseline: 1.0464x; 1.0464x over previous
"""Trainium2 Bass kernel for CausalGNNLayer (per-node-type Linear, MoE-style routing).

Semantics (matching the reference):
    out[n, :] = x[n, :] @ W[node_types[n]].T + b[node_types[n]]
edge_index is unused by the op.

Strategy (v2 — fp8e4 DoubleRow, P0-downclock-aware):
- Host-side routing-aware sharding: stable-sort nodes by type, split each
  type's node list into two halves -> 8 groups (4 types x 2 cores).
- Measured on this part: a sustained 8-core bf16-PE stream trips the chip's
  P0 power state and pins the PE at 2.0 GHz (259 ns / 512-wide matmul);
  an fp8e4 DoubleRow stream (2 MACs/PE/cycle, K=256 per instruction) stays
  at 2.4 GHz (216 ns).  DoubleRow needs both operands in fp8-e4m3, whose
  3-bit mantissa alone would blow the 2e-2 gate (measured 3.8e-2), so we
  run TWO DoubleRow streams accumulated in one PSUM group:
      y*2^17 = m1 @ Wq + u @ V,  where
      m1 = Q((1-a)*x*32),  xhat = m1/((1-a)*32),  xlo = x - xhat
      u  = Q((xlo + a*xhat)*32)
      Wq = Q(W*4096),      V = Q((What + (W-What)/a)*4096),  a = 1/8
  The correction stream cancels both operands' quantization error to first
  order: measured end-to-end rel err 7.2e-3 (vs 1.18e-2 for the old
  bf16xfp8e3 kernel).  Same instruction count as the bf16 schedule, but at
  216 ns/instr instead of 259: steady-state ~85 us vs ~102 us.
  Scales are powers of two (32*4096 = 2^17); bias is pre-scaled by 2^17 on
  host and the output divided by 2^17 after the run, so the device drain is
  still a plain add+downcast.
- Group-major tiles: 1024 nodes per x tile / psum burst / output tile, ONE
  dma_start in and ONE out per group (each dma_start costs ~700 ns of sync-
  queue DIRECT2D issue; the old 4-outs-per-group schedule kept the sync
  queue 94% busy and serialized the tail).
- out2 dram layout [128, 4, P] (partition-major) so the 4 psum drains of a
  group land in one SBUF tile and leave as one strided DMA.
- Warmup: dummy DoubleRow matmuls on a zeroed tile ramp the HAM clock gate
  (cold 1.2 GHz for ~3.4 us) while the first data DMAs land; mini 128-node
  first chunk starts real work on ~72 KB of data.
- Drain (bias add + fp32->bf16 downcast) alternates Vector/Scalar engines.
- Host scatters the 8 output shards back into the full [N, 512] fp32 output.
"""

import numpy as np
import ml_dtypes
from contextlib import ExitStack

import concourse.bass as bass
import concourse.mybir as mybir
import concourse.tile as tile
from concourse.bass_utils import run_bass_kernel_spmd

N_CORES = 8
IN_CH = 512
OUT_CH = 512
NUM_TYPES = 4
P_BLK = 128
OBLKS = OUT_CH // P_BLK   # 4
MINI_N = 128              # first chunk width (early compute start)
MID_N = 512               # second chunk width (bridges to the 1MB groups)
GRP_N = 1024              # steady group width (one x tile / one out tile)
XBUFS = 4                 # x group-tile prefetch depth (8KB/partition each)
PSBUFS = 4                # psum ring: 4 tiles x 2 banks = all 8 banks
OBUFS = 3                 # output staging depth (8KB/partition each)
TAIL_N = 256              # last chunk width (bounds the exposed final DMA)
WARMUP_MM = 12            # dummy DoubleRow matmuls to ramp the clock gate

ALPHA = 0.125
SM = 32.0                 # moving-operand scale
SW = 4096.0               # stationary-operand scale
SCALE = SM * SW           # 2^17

TRACE = False
LAST_RESULTS = None

_compile_cache: dict = {}

_legal_nop_counter = [0]


def _legalize_waits(nc: bass.Bass) -> None:
    """This walrus codegen only encodes ONE sync wait per engine instruction.
    Tile's scheduler attaches several.  Split: hoist all-but-one wait of any
    multi-wait instruction into preceding same-engine NoOps (one wait each) —
    semantically identical (the engine stalls on each wait in program order)."""
    for fn in nc.m.functions:
        for blk in fn.blocks:
            insts = blk.instructions
            out = []
            changed = False
            for inst in insts:
                si = inst.sync_info
                waits = list(si.on_wait) if si is not None and si.on_wait else []
                if len(waits) > 1:
                    changed = True
                    for w in waits[:-1]:
                        _legal_nop_counter[0] += 1
                        nop = mybir.InstNoOp(
                            name=f"waitsplit-{_legal_nop_counter[0]}",
                            ins=[],
                            outs=[],
                            engine=inst.engine,
                        )
                        nop.sync_info = mybir.SyncInfo(on_wait=[w], on_update=[])
                        out.append(nop)
                    inst.sync_info = mybir.SyncInfo(
                        on_wait=[waits[-1]], on_update=list(si.on_update or [])
                    )
                out.append(inst)
            if changed:
                blk.instructions = out


def _plan(P_needed: int):
    """Chunk widths [128, 512, 1024, ..., 1024, final] covering >= P_needed.
    Graded early chunks: the 128-node mini needs only 128KB of DMA (first
    data to land, ~13us after kernel start under 8-core HBM contention),
    the 512 bridges while the 1MB group tiles stream in."""
    rem = P_needed - MINI_N - MID_N - TAIL_N
    nfull = max(0, (rem - 1) // GRP_N)
    mid2 = rem - nfull * GRP_N
    mid2 = ((mid2 + 63) // 64) * 64
    # remainder group sits second-to-last; the fixed small TAIL_N group ends
    # the kernel so the last exposed out-DMA is only ~256KB.
    widths = (
        [MINI_N, MID_N] + [GRP_N] * nfull + ([mid2] if mid2 else []) + [TAIL_N]
    )
    offs = np.concatenate([[0], np.cumsum(widths)]).astype(int)
    return widths, list(offs[:-1]), int(offs[-1])


def _build_bass(plan_key) -> bass.Bass:
    widths, offs, P = plan_key
    nc = bass.Bass("TRN2")
    f32 = mybir.dt.float32
    bf16 = mybir.dt.bfloat16
    f8e4 = mybir.dt.float8e4
    DR = mybir.MatmulPerfMode.DoubleRow

    nchunks = len(widths)

    # xT[c, p, s, j, i, n]: stream s, kk-pair j, plane i, node n of chunk c;
    # contraction index kappa = j*256 + i*128 + p.
    xT = nc.dram_tensor(
        "xT", [nchunks, P_BLK, 2, 2, 2, GRP_N], f8e4, kind="ExternalInput"
    )
    # w8[p, s, j, i, o*128+m]
    w8 = nc.dram_tensor("w8", [P_BLK, 2, 2, 2, OUT_CH], f8e4, kind="ExternalInput")
    # bias2[p, oblk] = b[oblk*128 + p] * 2^17
    bias2 = nc.dram_tensor("bias2", [P_BLK, OBLKS], f32, kind="ExternalInput")
    # out2[p, oblk, n] = (y[n, oblk*128+p] * 2^17) as bf16
    out2 = nc.dram_tensor("out2", [P_BLK, OBLKS, P], bf16, kind="ExternalOutput")

    with ExitStack() as ctx:
        tc = ctx.enter_context(tile.TileContext(nc))
        sp = ctx.enter_context(tc.tile_pool(name="st", bufs=3))
        xp = ctx.enter_context(tc.tile_pool(name="x", bufs=XBUFS))
        pp = ctx.enter_context(tc.tile_pool(name="ps", bufs=PSBUFS, space="PSUM"))
        op = ctx.enter_context(tc.tile_pool(name="o", bufs=OBUFS))

        # Clock-gate warmup: dummy DoubleRow matmuls on zeros, no DMA deps.
        warm_sb = sp.tile([P_BLK, 2, 512], f8e4)
        nc.gpsimd.memset(warm_sb[:], 0)
        ps_warm = pp.tile([P_BLK, 512], f32, tag="ps")
        for _ in range(WARMUP_MM):
            nc.tensor.matmul(
                ps_warm[:],
                lhsT=warm_sb[:, :, 0:P_BLK],
                rhs=warm_sb[:],
                start=True,
                stop=True,
                perf_mode=DR,
            )

        x_tiles: dict[int, object] = {}

        def fetch_chunk(c: int):
            if c not in x_tiles:
                wd = widths[c]
                t = xp.tile([P_BLK, 2, 2, 2, wd], f8e4, tag="x")
                nc.sync.dma_start(t[:], xT.ap()[c][:, :, :, :, 0:wd])
                x_tiles[c] = t

        w_sb = sp.tile([P_BLK, 2, 2, 2, OUT_CH], f8e4)
        # x-in DMAs issue on the sync queue; w/bias/out on the scalar queue so
        # the two DIRECT2D streams (~0.7us each to issue) run in parallel and
        # the tail's final out-DMA doesn't queue behind x prefetches.
        nc.sync.dma_start(w_sb[:, 0], w8.ap()[:, 0])
        fetch_chunk(0)
        if len(widths) > 1:
            fetch_chunk(1)
        nc.sync.dma_start(w_sb[:, 1], w8.ap()[:, 1])
        b_sb = sp.tile([P_BLK, OBLKS], f32)
        nc.sync.dma_start(b_sb[:], bias2.ap())

        drain_flip = [0]

        def drain(o_sb, oblk, ps_ap, force_vector=False):
            bias_ap = b_sb[:, oblk: oblk + 1]
            if force_vector or drain_flip[0] % 2 == 0:
                nc.vector.tensor_scalar_add(o_sb[:, oblk, :], ps_ap, bias_ap)
            else:
                nc.scalar.add(o_sb[:, oblk, :], ps_ap, bias_ap)
            drain_flip[0] += 1

        for c in range(nchunks):
            fetch_chunk(c)
            for cn in range(c + 1, min(c + 3, nchunks)):
                fetch_chunk(cn)
            wd = widths[c]
            goff = offs[c]
            xt = x_tiles[c]
            o_sb = op.tile([P_BLK, OBLKS, wd], bf16, tag="o")
            if c == 0:
                # Mini chunk: (s,j)-outer over all 4 oblk psum slices so the
                # second w half gets extra time to land.  Slices sit at
                # 512-element offsets (psum-bank aligned).
                psA = pp.tile([P_BLK, 2 * 512], f32, tag="ps")
                psB = pp.tile([P_BLK, 2 * 512], f32, tag="ps")
                mslice = lambda oblk: (psA if oblk < 2 else psB)[
                    :, (oblk % 2) * 512: (oblk % 2) * 512 + wd
                ]
                for s in range(2):
                    for j in range(2):
                        for oblk in range(OBLKS):
                            nc.tensor.matmul(
                                mslice(oblk),
                                lhsT=w_sb[:, s, j, :, oblk * P_BLK:(oblk + 1) * P_BLK],
                                rhs=xt[:, s, j, :, 0:wd],
                                start=(s == 0 and j == 0),
                                stop=(s == 1 and j == 1),
                                perf_mode=DR,
                            )
                for oblk in range(OBLKS):
                    drain(o_sb, oblk, mslice(oblk))
            else:
                for oblk in range(OBLKS):
                    ps = pp.tile([P_BLK, wd], f32, tag="ps")
                    for s in range(2):
                        for j in range(2):
                            lhsT = w_sb[:, s, j, :, oblk * P_BLK:(oblk + 1) * P_BLK]
                            for h in range(0, wd, 512):
                                he = min(h + 512, wd)
                                nc.tensor.matmul(
                                    ps[:, h:he],
                                    lhsT=lhsT,
                                    rhs=xt[:, s, j, :, h:he],
                                    start=(s == 0 and j == 0),
                                    stop=(s == 1 and j == 1),
                                    perf_mode=DR,
                                )
                    drain(o_sb, oblk, ps[:])
            nc.sync.dma_start(out2.ap()[:, :, goff:goff + wd], o_sb[:])
    _legalize_waits(nc)
    return nc


def _get_compiled(plan_key) -> bass.Bass:
    key = (tuple(plan_key[0]), plan_key[2])
    if key not in _compile_cache:
        _compile_cache[key] = _build_bass(plan_key)
    return _compile_cache[key]


def _qe4(a):
    return np.clip(a, -224.0, 224.0).astype(ml_dtypes.float8_e4m3)


def kernel(x, edge_index, node_types, W, b):
    global LAST_RESULTS
    x = np.asarray(x, dtype=np.float32)
    nt = np.asarray(node_types).astype(np.int64)
    W = np.asarray(W, dtype=np.float32)
    b = np.asarray(b, dtype=np.float32)
    N = x.shape[0]

    # Route nodes: stable sort by type, split each type across 2 cores.
    order = np.argsort(nt, kind="stable")
    counts = np.bincount(nt, minlength=NUM_TYPES)
    shards = []
    start = 0
    for t in range(NUM_TYPES):
        c = int(counts[t])
        idx = order[start: start + c]
        start += c
        h = (c + 1) // 2
        shards.append(idx[:h])
        shards.append(idx[h:])

    P_needed = max(1, max(len(g) for g in shards))
    plan = _plan(P_needed)
    widths, offs, P = plan
    nchunks = len(widths)

    nc = _get_compiled(plan)

    # Per-type quantized weights (shared by the 2 cores of each type).
    w_packed = []
    b_packed = []
    for t in range(NUM_TYPES):
        Wq = _qe4(W[t] * SW)                      # [O, K] e4m3
        What = Wq.astype(np.float32) / SW
        V = _qe4((What + (W[t] - What) / ALPHA) * SW)
        # [p, s, j, i, o]: Wq/V [o, kappa] -> kappa = j*256 + i*128 + p
        wp = np.empty((P_BLK, 2, 2, 2, OUT_CH), ml_dtypes.float8_e4m3)
        for s, Ws in enumerate((Wq, V)):
            # Ws.T [K, O] -> [j, i, p, O]
            wt = Ws.T.reshape(2, 2, P_BLK, OUT_CH)
            wp[:, s] = wt.transpose(2, 0, 1, 3)
        w_packed.append(wp)
        b_packed.append(
            np.ascontiguousarray((b[t] * SCALE).reshape(OBLKS, P_BLK).T.astype(np.float32))
        )

    in_maps = []
    for gi, g in enumerate(shards):
        t = gi // 2
        xs = np.zeros((P, IN_CH), np.float32)
        if len(g):
            xs[: len(g)] = x[g]
        m1 = _qe4((1 - ALPHA) * xs * SM)
        xhat = m1.astype(np.float32) / ((1 - ALPHA) * SM)
        u = _qe4((xs - xhat + ALPHA * xhat) * SM)
        xbuf = np.zeros((nchunks, P_BLK, 2, 2, 2, GRP_N), ml_dtypes.float8_e4m3)
        for s, Xs in enumerate((m1, u)):
            XsT = Xs.T.reshape(2, 2, P_BLK, P)  # [j, i, p, node]
            for c in range(nchunks):
                wd = widths[c]
                xbuf[c, :, s, :, :, :wd] = XsT[
                    :, :, :, offs[c]: offs[c] + wd
                ].transpose(2, 0, 1, 3)
        in_maps.append({"xT": xbuf, "w8": w_packed[t], "bias2": b_packed[t]})

    res = run_bass_kernel_spmd(nc, in_maps, list(range(N_CORES)), trace=TRACE)
    LAST_RESULTS = res

    inv = np.float32(1.0 / SCALE)
    out = np.empty((N, OUT_CH), np.float32)
    for gi, g in enumerate(shards):
        if len(g):
            # out2 [128, 4, P] -> [P, 512] node-major (channel = oblk*128+p)
            o = res.results[gi]["out2"].astype(np.float32).transpose(2, 1, 0)
            out[g] = o.reshape(P, OUT_CH)[: len(g)] * inv
    return out
